# revision 20
# baseline (speedup 1.0000x reference)
"""NT-Xent loss kernel for Trainium2, distributed across 8 NeuronCores.

v2: symmetric-pair scheme. The sim matrix is symmetric, so each unordered
pair {i,j} is computed ONCE globally; the exp value contributes to row i's
sum locally (ACT accum) and to row j's sum via a column-sum that is shipped
to j's owner core through an AllGather at the end.

Per core c (inputs rolled by 1024*c rows so the program is pure SPMD; only
rows 0..5119 local are loaded -- cols beyond block-distance 4 are owned by
other cores):
  - prep: load x (5 groups), norms via exp(-0.5*ln(sum x^2)), normalize
    (DVE), PE-transpose into xnT [128(d), 5120(rows)] (bf16)
  - main loop over 8 row Mtiles of my 1024 rows:
      blocks at col distance d=1,2,3 (full weight), d=0 diag (full), d=4
      (half weight: pairs at distance 4 are computed by BOTH end cores).
      PE: 10 [128,512]-matmuls (bf16 PSUM granules), ACT: 2 exp instrs
      ([3072] d123 + 2-run [2048] d0|d4) with accum_out row-sums.
      Col-sums for d=1..4: per-128-col chunk, lhsT=e-chunk, rhs=ones
      (halfones for d4) accumulated in a persistent PSUM bank cs[128,32].
  - d4 row-sums (to be halved) come from one DVE reduce over the kept e04
    tiles.
  - exchange: cs -> SBUF -> DRAM -> AllGather(8) -> [128,8,32] -> masked
    reduce with a per-core 0/1 mask input selects the 4 incoming vectors
    (from cores c-1..c-4) -> remote[128,8].
  - rowtotal = rsA + rsB - 0.5*d4sum + remote; ln(rowtotal - e^2) summed,
    minus 2*sum(pos-pair sims); partition-reduce via ones-matmul -> scalar.
Host sums the 8 partial scalars.

Matmult instructions can carry only ONE sync wait: all matmul deps are
arranged to come from ACT's semaphore (xnT copies + PSUM frees by ACT
consumers); colsum matmul deps come from ACT's e writes.
"""

import numpy as np

import concourse.bass as bass
import concourse.tile as tile
from concourse import mybir
from concourse.bass_utils import run_bass_kernel_spmd
from concourse.masks import make_identity

N2 = 8192           # total rows (2N)
D = 128             # feature dim
NCORES = 8
RPC = N2 // NCORES  # rows per core = 1024
NLOAD = 5 * RPC     # rows loaded per core (distances 0..4) = 5120
NCH = NLOAD // 128  # 40 chunks of 128 rows
NGRP = NCH // 8     # 5 DMA groups
F32 = mybir.dt.float32
BF16 = mybir.dt.bfloat16
AF = mybir.ActivationFunctionType
ALU = mybir.AluOpType
E2 = float(np.exp(2.0, dtype=np.float64))  # diag term exp(sim_ii / T), T=0.5

# PSUM: 8 banks x 512 f32 per partition.  1 bank holds the col-sum
# accumulators; the other 7 are a ring of [128,512] sim slots.  Each Mtile
# makes 10 slot-writes (d1..d3: 6, d0: 2, d4: 2) rotating by 3 slots per
# Mtile; d123 is consumed by 1-2 contiguous-run ACT instrs, d0/d4 by
# stepped-pair ACT instrs.
NSLOT = 7


def _slot_plan(m):
    """Returns (d123_slots_sorted, d0_pair_sorted, d4_pair_sorted)."""
    w = [(3 * m + i) % NSLOT for i in range(10)]
    return sorted(w[0:6]), sorted(w[6:8]), sorted(w[8:10])


def _runs(slots):
    """Split sorted slot list into maximal contiguous runs."""
    runs = [[slots[0]]]
    for s in slots[1:]:
        if s == runs[-1][-1] + 1:
            runs[-1].append(s)
        else:
            runs.append([s])
    return runs


def _emit(tc: tile.TileContext, ctx, out_ap: bass.AP, x_ap: bass.AP,
          mask_ap: bass.AP, dbg_aps=None):
    nc = tc.nc

    big = ctx.enter_context(tc.tile_pool(name="big", bufs=1))
    esc = ctx.enter_context(tc.tile_pool(name="esc", bufs=2))
    small = ctx.enter_context(tc.tile_pool(name="small", bufs=1))
    dram = ctx.enter_context(tc.tile_pool(name="dram", bufs=1, space="DRAM"))

    # one tile per DMA group: keeps each consumer waiting on a single DMA sem
    x_g = [
        big.tile([128, 8, 128], F32, tag=f"x{g}", name=f"x_{g}")
        for g in range(NGRP)
    ]
    xsq_g = [
        big.tile([128, 8, 128], F32, tag=f"xsq{g}", name=f"xsq_{g}")
        for g in range(NGRP)
    ]
    xb = big.tile([128, NCH, 128], BF16, tag="xb")      # normalized, bf16
    # transposed normalized matrix: cols [0,2048),[2048,4096),[4096,5120)
    xnT0 = big.tile([128, 2048], BF16, tag="xnT0")
    xnT1 = big.tile([128, 2048], BF16, tag="xnT1")
    xnT2 = big.tile([128, 1024], BF16, tag="xnT2")

    s = small.tile([128, NCH], F32)      # squared norms
    ls = small.tile([128, NCH], F32)
    r = small.tile([128, NCH], F32)      # 1/norm
    r_dve = small.tile([128, NCH], F32)  # DVE-local copy
    iprobe = small.tile([1, 1], BF16)    # DVE probe of ident (Pool->DVE edge)
    mprobe = small.tile([1, 1], F32)     # DVE probe of mask DMA
    rsA = small.tile([128, 16], F32)     # accum: d1+d2+d3 (2 cols per Mtile)
    rsA2 = small.tile([128, 8], F32)
    rsB = small.tile([128, 8], F32)      # accum: d0 row sums
    rsC = small.tile([128, 8], F32)      # accum: d4 row sums (to be halved)
    csb = small.tile([128, 32], F32)     # col sums, SBUF copy
    csb1 = small.tile([128, 32], F32)
    csb2 = small.tile([128, 32], F32)
    gg = small.tile([128, 8, 32], F32)   # AllGather result
    gm = small.tile([128, 256], F32)     # masked gather
    mask_t = small.tile([128, 32], F32)  # per-core 0/1 mask input
    remote = small.tile([128, 8], F32)   # summed remote contributions
    rt = small.tile([128, 8], F32)
    rt2 = small.tile([128, 8], F32)
    rt3 = small.tile([128, 8], F32)
    lg = small.tile([128, 8], F32)
    logsum = small.tile([128, 1], F32)
    possum = small.tile([128, 1], F32)
    fin = small.tile([128, 1], F32)
    fin2 = small.tile([128, 1], F32)     # ACT-written copy (matmul 1-wait)
    ones = small.tile([128, 1], F32)     # ACT-written (final reduce)
    onesb = small.tile([128, 1], BF16)   # ACT-written (colsum rhs)
    halfb = small.tile([128, 1], BF16)   # ACT-written (d4 colsum rhs)
    ident = small.tile([128, 128], BF16)
    fin_sb = small.tile([1, 1], F32)
    pos_scr = small.tile([128, RPC], BF16)
    negE2 = small.tile([128, 1], F32)

    ag_in = dram.tile([128, 32], F32, tag="ag_in")
    ag_out = dram.tile([1024, 32], F32, tag="ag_out")

    nc.vector.memset(negE2, -E2)
    make_identity(nc, ident)
    # DVE probe-read of ident: every later DVE op transitively implies the
    # identity is built, letting the strip pass drop Pool waits from the
    # transpose matmuls (which can carry only one sync wait).
    nc.vector.tensor_copy(iprobe, ident[0:1, 0:1])
    # constants written by ACT so consuming matmuls wait on ACT only
    nc.scalar.activation(out=ones, in_=negE2, func=AF.Copy, bias=1.0, scale=0.0)
    nc.scalar.activation(out=onesb, in_=negE2, func=AF.Copy, bias=1.0, scale=0.0)
    nc.scalar.activation(out=halfb, in_=negE2, func=AF.Copy, bias=0.5, scale=0.0)

    nc.sync.dma_start(out=mask_t, in_=mask_ap)
    # DVE probe-read: later DVE ops transitively imply the mask has arrived
    nc.vector.tensor_copy(mprobe, mask_t[0:1, 0:1])

    x_src = x_ap.rearrange("(c p) d -> p c d", p=128)

    # ---- prep: load + norms + normalize + transpose, pipelined per group ----
    with tc.tile_pool(name="prep_ps", bufs=1, space="PSUM") as prep_ps:
        pt0 = prep_ps.tile([128, 2048], BF16, tag="pt0")
        pt1 = prep_ps.tile([128, 2048], BF16, tag="pt1")
        pt2 = prep_ps.tile([128, 1024], BF16, tag="pt2")
        pts = [pt0, pt1, pt2]
        for g in range(NGRP):
            sl = slice(8 * g, 8 * g + 8)
            nc.sync.dma_start(out=x_g[g][:, :, :], in_=x_src[:, sl, :])
            nc.vector.tensor_mul(
                xsq_g[g][:, :, :], x_g[g][:, :, :], x_g[g][:, :, :]
            )
            nc.vector.tensor_reduce(
                out=s[:, sl],
                in_=xsq_g[g][:, :, :],
                axis=mybir.AxisListType.X,
                op=ALU.add,
            )
            # r = exp(-0.5*ln(s)) == s^-1/2 ; exp+ln share one ACT table set
            nc.scalar.activation(out=ls[:, sl], in_=s[:, sl], func=AF.Ln)
            nc.scalar.activation(
                out=r[:, sl], in_=ls[:, sl], func=AF.Exp, scale=-0.5
            )
            nc.vector.tensor_copy(r_dve[:, sl], r[:, sl])
            for c in range(8 * g, 8 * g + 8):
                nc.vector.tensor_scalar_mul(
                    out=xb[:, c, :],
                    in0=x_g[c // 8][:, c % 8, :],
                    scalar1=r_dve[:, c : c + 1],
                )
            # transpose completed pairs of groups (and the final odd group)
            if g % 2 == 1 or g == NGRP - 1:
                tg = g // 2
                pt = pts[tg]
                nchunks = 16 if g % 2 == 1 else 8
                for k in range(nchunks):
                    ch = 16 * tg + k
                    nc.tensor.transpose(
                        pt[:, 128 * k : 128 * (k + 1)], xb[:, ch, :], ident
                    )
                # copy on ACT: matmuls consuming xnT then wait on ACT only
                if tg == 0:
                    nc.scalar.copy(xnT0[:, :], pt[:, :])
                elif tg == 1:
                    nc.scalar.copy(xnT1[:, :], pt[:, :])
                else:
                    nc.scalar.copy(xnT2[:, :], pt[:, :])

    def xnT_col(j, width):
        """AP for xnT columns [j, j+width) -- must stay in one tile."""
        if j < 2048:
            assert j + width <= 2048
            return xnT0[:, j : j + width]
        if j < 4096:
            assert j + width <= 4096
            return xnT1[:, j - 2048 : j - 2048 + width]
        assert j + width <= 5120
        return xnT2[:, j - 4096 : j - 4096 + width]

    # ---- main loop ----
    ps = ctx.enter_context(tc.tile_pool(name="ps", bufs=1, space="PSUM"))
    ring = ps.tile([128, NSLOT, 512], F32, tag="ring")  # 7 banks
    # col-sum scratch: one [128,1] slot per (mtile, d, chunk); every matmul
    # is start&stop (PSUM allows only one PENDING accumulation group per
    # bank zero-region, so cross-instruction accumulation is illegal here)
    scr = ps.tile([128, 8, 32], F32, tag="scr")         # 1KB of bank 7

    nc.vector.memset(rsA, 0.0)

    def pair_ap(pr):
        """[128,2,512] AP over an arbitrary sorted slot pair."""
        u, v = pr
        if v == u + 1:
            return ring[:, u : u + 2, :]
        return ring[:, u : v + 1 : v - u, :]

    e123_t = []
    e4_t = []
    e0scrap = small.tile([128, 1024], BF16, tag="e0scrap")
    for m in range(8):
        s123, p0, p4 = _slot_plan(m)
        lhsT = xnT0[:, 128 * m : 128 * (m + 1)]
        # content by ascending slot address: d1 -> s123[0:2], d2 -> [2:4],
        # d3 -> [4:6]; matching e123 cols [0:1024) [1024:2048) [2048:3072)
        for i, sl in enumerate(s123):
            nc.tensor.matmul(
                ring[:, sl, :],
                lhsT=lhsT,
                rhs=xnT_col(1024 + 512 * i, 512),
                start=True,
                stop=True,
            )
        # ACT exp passes; T=0.5 and f32 sim -> scale=2.0.  The d0/d4
        # matmuls reuse d123 slots, so they MUST be emitted after ACT-A.
        e123 = esc.tile([128, 3072], BF16, tag="e123", name=f"e123_{m}")
        e123_t.append(e123)
        off = 0
        for k, run in enumerate(_runs(s123)):
            w = 512 * len(run)
            nc.scalar.activation(
                out=e123[:, off : off + w].rearrange(
                    "p (a b) -> p a b", b=512
                ),
                in_=ring[:, run[0] : run[0] + len(run), :],
                func=AF.Exp,
                scale=2.0,
                accum_out=rsA[:, 2 * m + k : 2 * m + k + 1],
            )
            off += w
        for i, sl in enumerate(p0):
            nc.tensor.matmul(
                ring[:, sl, :], lhsT=lhsT, rhs=xnT_col(512 * i, 512),
                start=True, stop=True,
            )
        for i, sl in enumerate(p4):
            nc.tensor.matmul(
                ring[:, sl, :], lhsT=lhsT, rhs=xnT_col(4096 + 512 * i, 512),
                start=True, stop=True,
            )
        nc.scalar.activation(
            out=e0scrap.rearrange("p (a b) -> p a b", b=512),
            in_=pair_ap(p0), func=AF.Exp, scale=2.0,
            accum_out=rsB[:, m : m + 1],
        )
        e4 = esc.tile([128, 1024], BF16, tag="e4", name=f"e4_{m}")
        e4_t.append(e4)
        nc.scalar.activation(
            out=e4.rearrange("p (a b) -> p a b", b=512),
            in_=pair_ap(p4), func=AF.Exp, scale=2.0,
            accum_out=rsC[:, m : m + 1],
        )
        if m == 1 and dbg_aps is not None and "e123" in dbg_aps:
            nc.gpsimd.dma_start(out=dbg_aps["e123"], in_=e123_t[0])
            nc.gpsimd.dma_start(out=dbg_aps["e4"], in_=e4_t[0])
        # col sums of the previous Mtile's e tiles (e ready; PE waits on ACT)
        if m > 0:
            _emit_colsums(nc, e123_t[m - 1], e4_t[m - 1], m - 1, scr,
                          onesb, halfb)
        if m == 5:
            # fold Mtiles 0-3 early so the tail only reduces 4-7
            nc.vector.tensor_reduce(
                out=csb1,
                in_=scr[:, 0:4, :].rearrange("p m c -> p c m"),
                axis=mybir.AxisListType.X,
                op=ALU.add,
            )
    _emit_colsums(nc, e123_t[7], e4_t[7], 7, scr, onesb, halfb)
    nc.vector.tensor_reduce(
        out=csb2,
        in_=scr[:, 4:8, :].rearrange("p m c -> p c m"),
        axis=mybir.AxisListType.X,
        op=ALU.add,
    )
    nc.vector.tensor_tensor(out=csb, in0=csb1, in1=csb2, op=ALU.add)

    # rsA2[p,m] = rsA[p,2m] + rsA[p,2m+1]
    nc.vector.tensor_reduce(
        out=rsA2,
        in_=rsA.rearrange("p (m k) -> p m k", k=2),
        axis=mybir.AxisListType.X,
        op=ALU.add,
    )

    # ---- positive-pair term: sum over my rows of sim(i, i+N) ----
    # local pos column of local row i is always i + 4096 (rotation invariant)
    nc.vector.tensor_mul(pos_scr, xnT0[:, 0:RPC], xnT2[:, 0:RPC])
    nc.vector.tensor_reduce(
        out=possum, in_=pos_scr, axis=mybir.AxisListType.X, op=ALU.add
    )

    # ---- exchange: csb -> DRAM -> AllGather -> masked reduce ----
    nc.sync.dma_start(out=ag_in[:, :], in_=csb)
    nc.gpsimd.collective_compute(
        "AllGather",
        ALU.bypass,
        replica_groups=[list(range(NCORES))],
        ins=[ag_in[:, :].opt()],
        outs=[ag_out[:, :].opt()],
    )
    nc.sync.dma_start(
        out=gg, in_=ag_out[:, :].rearrange("(s p) c -> p s c", p=128)
    )
    # gm[p, s, d, ch] = gg * mask (mask broadcast along ch)
    nc.vector.tensor_mul(
        gm.rearrange("p (s d ch) -> p s d ch", s=8, d=4),
        gg.rearrange("p s (d ch) -> p s d ch", d=4),
        mask_t.rearrange("p (s d) -> p s d", s=8).broadcast_to([128, 8, 4, 8]),
    )
    # remote[p, m] = sum over (s,d) of gm  (s,d flatten to stride-8 x 32)
    nc.vector.tensor_reduce(
        out=remote,
        in_=gm.rearrange("p (sd ch) -> p ch sd", ch=8),
        axis=mybir.AxisListType.X,
        op=ALU.add,
    )

    # ---- finals ----
    nc.vector.tensor_tensor(out=rt, in0=rsA2, in1=rsB, op=ALU.add)
    # rt2 = rt + 0.5*rsC  (d4 blocks are computed by both end cores)
    nc.vector.scalar_tensor_tensor(
        out=rt2, in0=rsC, scalar=0.5, in1=rt, op0=ALU.mult, op1=ALU.add
    )
    nc.vector.tensor_tensor(out=rt3, in0=rt2, in1=remote, op=ALU.add)
    # lg = ln(rowtotal - e^2), logsum = sum over the 8 Mtiles
    nc.scalar.activation(
        out=lg, in_=rt3, func=AF.Ln, bias=negE2[:, :], scale=1.0,
        accum_out=logsum,
    )
    # fin = logsum - 2 * possum
    nc.vector.scalar_tensor_tensor(
        out=fin,
        in0=possum,
        scalar=-2.0,
        in1=logsum,
        op0=ALU.mult,
        op1=ALU.add,
    )
    nc.scalar.copy(fin2, fin)  # ACT hop: final matmul waits on ACT only
    # partition reduce via ones-matmul into a retired ring bank
    pf = ring[:, 0, :]
    nc.tensor.matmul(pf[0:1, 0:1], lhsT=fin2, rhs=ones, start=True, stop=True)
    nc.vector.tensor_copy(fin_sb, pf[0:1, 0:1])
    # SWDGE for the tiny output write (direct-2D carries only one sync wait)
    nc.gpsimd.dma_start(out=out_ap, in_=fin_sb)
    if dbg_aps is not None:
        dbg_aps = dict(dbg_aps)
        stage = small.tile([128, 73], F32, tag="dbgstage")
        off = 0
        for t, w in [(rsA2, 8), (rsB, 8), (rsC, 8), (csb, 32), (remote, 8),
                     (rt3, 8), (possum, 1)]:
            nc.vector.tensor_copy(stage[:, off : off + w], t)
            off += w
        nc.gpsimd.dma_start(out=dbg_aps["stage"], in_=stage)


def _emit_colsums(nc, e123, e4, m, scr, onesb, halfb):
    """Column sums via lhsT=e-chunk, rhs=ones; one closed matmul per slot."""
    for d in range(3):          # d = 1,2,3 from e123
        for ch in range(8):
            nc.tensor.matmul(
                scr[:, m, 8 * d + ch : 8 * d + ch + 1],
                lhsT=e123[:, 1024 * d + 128 * ch : 1024 * d + 128 * (ch + 1)],
                rhs=onesb,
                start=True,
                stop=True,
            )
    for ch in range(8):         # d = 4 (half weight)
        nc.tensor.matmul(
            scr[:, m, 24 + ch : 24 + ch + 1],
            lhsT=e4[:, 128 * ch : 128 * (ch + 1)],
            rhs=halfb,
            start=True,
            stop=True,
        )


def _strip_self_waits(nc):
    """Drop engine-self semaphore waits from Matmult/Activation instructions.

    PE and ACT are strict in-order single queues whose semaphores increment
    at instruction completion in program order, so a wait on the engine's own
    semaphore is always transitively implied by queue order.  The Matmult
    instruction encoding only has room for ONE sync wait, so the extra
    self-wait breaks walrus codegen ("Too many sync wait commands").
    """
    eng_prefix = {
        mybir.EngineType.PE: "PE_",
        mybir.EngineType.Activation: "Activation_",
        mybir.EngineType.DVE: "DVE_",
    }
    for bb in nc.main_func.blocks:
        for ins in bb.instructions:
            si = ins.sync_info
            if si is None:
                continue
            if type(ins).__name__ == "InstDrain":
                # The tail drain's encoding carries only ONE sync wait.  The
                # output DMA's completion (DMASW, last in program order)
                # transitively implies the rest in the shipping build; debug
                # builds accept a rare stale 'out' in exchange for compiling.
                w = list(si.on_wait)
                sw = [x for x in w if (x.ant_name or "").startswith("DMASW")]
                if len(w) > 1 and sw:
                    si.on_wait = [sw[-1]]
                continue
            pfx = eng_prefix.get(getattr(ins, "engine", None))
            if pfx is None:
                continue
            w = list(si.on_wait)
            w2 = [x for x in w if not (x.ant_name or "").startswith(pfx)]
            if type(ins).__name__ == "InstMatmult":
                # Pool only produces the identity matrix here, and the DVE
                # probe-read of it precedes every DVE-produced matmul input,
                # so any Pool wait on a matmul is transitively covered by
                # its DVE wait.
                w2 = [x for x in w2 if not (x.ant_name or "").startswith("Pool_")]
            if len(w2) != len(w):
                si.on_wait = w2


def _build(strip: bool = True, debug_taps: bool = False):
    from contextlib import ExitStack

    nc = bass.Bass("TRN2", debug=False, num_devices=NCORES)
    x_in = nc.dram_tensor("x", [NLOAD, D], F32, kind="ExternalInput")
    mask_in = nc.dram_tensor("mask", [128, 32], F32, kind="ExternalInput")
    out = nc.dram_tensor("out", [1, 1], F32, kind="ExternalOutput")
    dbg_aps = None
    if debug_taps:
        dbg_aps = {
            "stage": nc.dram_tensor(
                "dbg", [128, 73], F32, kind="ExternalOutput"
            ).ap(),
            "e123": nc.dram_tensor(
                "dbg_e123", [128, 3072], BF16, kind="ExternalOutput"
            ).ap(),
            "e4": nc.dram_tensor(
                "dbg_e4", [128, 1024], BF16, kind="ExternalOutput"
            ).ap(),
        }
    with tile.TileContext(nc) as tc:
        with ExitStack() as ctx:
            _emit(tc, ctx, out.ap(), x_in.ap(), mask_in.ap(), dbg_aps)
    if strip:
        _strip_self_waits(nc)
    return nc


_NC_CACHE = None
_WARMED = False


def _get_nc():
    global _NC_CACHE
    if _NC_CACHE is None:
        _NC_CACHE = _build()
    return _NC_CACHE


def warmup(nc, in_maps):
    """First-ever execution races the NRT collective-comm init (the
    AllGather's data can land after its semaphore fires), so prime the
    comm path once; all warm executions are fully synchronized."""
    global _WARMED
    if not _WARMED:
        run_bass_kernel_spmd(nc, in_maps, core_ids=list(range(NCORES)))
        _WARMED = True


def make_in_maps(x: np.ndarray) -> list[dict]:
    """Per-core inputs: rolled+truncated x and the exchange-select mask."""
    in_maps = []
    for c in range(NCORES):
        xr = np.ascontiguousarray(
            np.roll(x, -RPC * c, axis=0)[:NLOAD], dtype=np.float32
        )
        mask = np.zeros((8, 4), dtype=np.float32)
        for d in range(1, 5):
            mask[(c - d) % NCORES, d - 1] = 1.0
        mask_t = np.ascontiguousarray(
            np.broadcast_to(mask.reshape(1, 32), (128, 32)), dtype=np.float32
        )
        in_maps.append({"x": xr, "mask": mask_t})
    return in_maps


def kernel(**inputs) -> np.ndarray:
    x = np.ascontiguousarray(
        np.asarray(inputs["projected_vectors"]), dtype=np.float32
    )
    assert x.shape == (N2, D)
    nc = _get_nc()
    in_maps = make_in_maps(x)
    warmup(nc, in_maps)
    res = run_bass_kernel_spmd(nc, in_maps, core_ids=list(range(NCORES)))
    total = np.float32(0.0)
    for rmap in res.results:
        total += np.float32(rmap["out"][0, 0])
    return np.asarray(total, dtype=np.float32)


if __name__ == "__main__":
    xt = np.random.randn(N2, D).astype(np.float32)
    print(kernel(projected_vectors=xt))


# revision 22
# speedup vs baseline: 1.2338x; 1.2338x over previous
"""NT-Xent loss kernel for Trainium2, distributed across 8 NeuronCores.

v2: symmetric-pair scheme. The sim matrix is symmetric, so each unordered
pair {i,j} is computed ONCE globally; the exp value contributes to row i's
sum locally (ACT accum) and to row j's sum via a column-sum that is shipped
to j's owner core through an AllGather at the end.

Per core c (inputs rolled by 1024*c rows so the program is pure SPMD; only
rows 0..5119 local are loaded -- cols beyond block-distance 4 are owned by
other cores):
  - prep: load x (5 groups), norms via exp(-0.5*ln(sum x^2)), normalize
    (DVE), PE-transpose into xnT [128(d), 5120(rows)] (bf16)
  - main loop over 8 row Mtiles of my 1024 rows:
      blocks at col distance d=1,2,3 (full weight), d=0 diag (full), d=4
      (half weight: pairs at distance 4 are computed by BOTH end cores).
      PE: 10 [128,512]-matmuls (bf16 PSUM granules), ACT: 2 exp instrs
      ([3072] d123 + 2-run [2048] d0|d4) with accum_out row-sums.
      Col-sums for d=1..4: per-128-col chunk, lhsT=e-chunk, rhs=ones
      (halfones for d4) accumulated in a persistent PSUM bank cs[128,32].
  - d4 row-sums (to be halved) come from one DVE reduce over the kept e04
    tiles.
  - exchange: cs -> SBUF -> DRAM -> AllGather(8) -> [128,8,32] -> masked
    reduce with a per-core 0/1 mask input selects the 4 incoming vectors
    (from cores c-1..c-4) -> remote[128,8].
  - rowtotal = rsA + rsB - 0.5*d4sum + remote; ln(rowtotal - e^2) summed,
    minus 2*sum(pos-pair sims); partition-reduce via ones-matmul -> scalar.
Host sums the 8 partial scalars.

Matmult instructions can carry only ONE sync wait: all matmul deps are
arranged to come from ACT's semaphore (xnT copies + PSUM frees by ACT
consumers); colsum matmul deps come from ACT's e writes.
"""

import numpy as np

import concourse.bass as bass
import concourse.tile as tile
from concourse import mybir
from concourse.bass_utils import run_bass_kernel_spmd
from concourse.masks import make_identity

N2 = 8192           # total rows (2N)
D = 128             # feature dim
NCORES = 8
RPC = N2 // NCORES  # rows per core = 1024
NLOAD = 5 * RPC     # rows loaded per core (distances 0..4) = 5120
NCH = NLOAD // 128  # 40 chunks of 128 rows
NGRP = NCH // 8     # 5 DMA groups
F32 = mybir.dt.float32
BF16 = mybir.dt.bfloat16
AF = mybir.ActivationFunctionType
ALU = mybir.AluOpType
E2 = float(np.exp(2.0, dtype=np.float64))  # diag term exp(sim_ii / T), T=0.5

# PSUM: 8 banks x 512 f32 per partition.  1 bank holds the col-sum
# accumulators; the other 7 are a ring of [128,512] sim slots.  Each Mtile
# makes 10 slot-writes (d1..d3: 6, d0: 2, d4: 2) rotating by 3 slots per
# Mtile; d123 is consumed by 1-2 contiguous-run ACT instrs, d0/d4 by
# stepped-pair ACT instrs.
NSLOT = 7


def _slot_plan(m):
    """Returns (d123_slots_sorted, d0_pair_sorted, d4_pair_sorted)."""
    w = [(3 * m + i) % NSLOT for i in range(10)]
    return sorted(w[0:6]), sorted(w[6:8]), sorted(w[8:10])


def _runs(slots):
    """Split sorted slot list into maximal contiguous runs."""
    runs = [[slots[0]]]
    for s in slots[1:]:
        if s == runs[-1][-1] + 1:
            runs[-1].append(s)
        else:
            runs.append([s])
    return runs


def _emit(tc: tile.TileContext, ctx, out_ap: bass.AP, x_ap: bass.AP,
          mask_ap: bass.AP, dbg_aps=None):
    nc = tc.nc

    big = ctx.enter_context(tc.tile_pool(name="big", bufs=1))
    esc = ctx.enter_context(tc.tile_pool(name="esc", bufs=2))
    small = ctx.enter_context(tc.tile_pool(name="small", bufs=1))
    dram = ctx.enter_context(tc.tile_pool(name="dram", bufs=1, space="DRAM"))

    # one tile per DMA group: keeps each consumer waiting on a single DMA sem
    x_g = [
        big.tile([128, 8, 128], F32, tag=f"x{g}", name=f"x_{g}")
        for g in range(NGRP)
    ]
    xsq_g = [
        big.tile([128, 8, 128], F32, tag=f"xsq{g}", name=f"xsq_{g}")
        for g in range(NGRP)
    ]
    xb = big.tile([128, NCH, 128], BF16, tag="xb")      # normalized, bf16
    # transposed normalized matrix: cols [0,2048),[2048,4096),[4096,5120)
    xnT0 = big.tile([128, 2048], BF16, tag="xnT0")
    xnT1 = big.tile([128, 2048], BF16, tag="xnT1")
    xnT2 = big.tile([128, 1024], BF16, tag="xnT2")
    # e tiles are all retained: col-sums and the d4 row-sum fix read them late
    eA = big.tile([128, 8, 2048], BF16, tag="eA")
    eB = big.tile([128, 8, 2048], BF16, tag="eB")
    e4a = big.tile([128, 8, 1024], BF16, tag="e4a")

    s = small.tile([128, NCH], F32)      # squared norms
    ls = small.tile([128, NCH], F32)
    r = small.tile([128, NCH], F32)      # 1/norm
    r_dve = small.tile([128, NCH], F32)  # DVE-local copy
    iprobe = small.tile([1, 1], BF16)    # DVE probe of ident (Pool->DVE edge)
    mprobe = small.tile([1, 1], F32)     # DVE probe of mask DMA
    rsAB = small.tile([128, 16], F32)    # accum: phase A (0:8), B (8:16)
    rsA2 = small.tile([128, 8], F32)
    rsC = small.tile([128, 8], F32)      # accum: d4 row sums (to be halved)
    csb = small.tile([128, 32], F32)     # col sums (DVE-folded)
    gg = small.tile([128, 8, 32], F32)   # AllGather result
    gm = small.tile([128, 256], F32)     # masked gather
    mask_t = small.tile([128, 32], F32)  # per-core 0/1 mask input
    remote = small.tile([128, 8], F32)   # summed remote contributions
    rt = small.tile([128, 8], F32)
    rt2 = small.tile([128, 8], F32)
    rt3 = small.tile([128, 8], F32)
    lg = small.tile([128, 8], F32)
    logsum = small.tile([128, 1], F32)
    possum = small.tile([128, 1], F32)
    fin = small.tile([128, 1], F32)
    fin2 = small.tile([128, 1], F32)     # ACT-written copy (matmul 1-wait)
    ones = small.tile([128, 1], F32)     # ACT-written (final reduce)
    onesb = small.tile([128, 1], BF16)   # ACT-written (colsum rhs)
    halfb = small.tile([128, 1], BF16)   # ACT-written (d4 colsum rhs)
    ident = small.tile([128, 128], BF16)
    fin_sb = small.tile([1, 1], F32)
    pos_scr = small.tile([128, RPC], BF16)
    negE2 = small.tile([128, 1], F32)

    ag_in = dram.tile([128, 32], F32, tag="ag_in")
    ag_out = dram.tile([1024, 32], F32, tag="ag_out")

    nc.vector.memset(negE2, -E2)
    make_identity(nc, ident)
    # DVE probe-read of ident: every later DVE op transitively implies the
    # identity is built, letting the strip pass drop Pool waits from the
    # transpose matmuls (which can carry only one sync wait).
    nc.vector.tensor_copy(iprobe, ident[0:1, 0:1])
    # constants written by ACT so consuming matmuls wait on ACT only
    nc.scalar.activation(out=ones, in_=negE2, func=AF.Copy, bias=1.0, scale=0.0)
    nc.scalar.activation(out=onesb, in_=negE2, func=AF.Copy, bias=1.0, scale=0.0)
    nc.scalar.activation(out=halfb, in_=negE2, func=AF.Copy, bias=0.5, scale=0.0)

    nc.sync.dma_start(out=mask_t, in_=mask_ap)
    # DVE probe-read: later DVE ops transitively imply the mask has arrived
    nc.vector.tensor_copy(mprobe, mask_t[0:1, 0:1])

    x_src = x_ap.rearrange("(c p) d -> p c d", p=128)

    # ---- prep: load + norms + normalize + transpose, pipelined per group ----
    with tc.tile_pool(name="prep_ps", bufs=1, space="PSUM") as prep_ps:
        pt0 = prep_ps.tile([128, 2048], BF16, tag="pt0")
        pt1 = prep_ps.tile([128, 2048], BF16, tag="pt1")
        pt2 = prep_ps.tile([128, 1024], BF16, tag="pt2")
        pts = [pt0, pt1, pt2]
        for g in range(NGRP):
            sl = slice(8 * g, 8 * g + 8)
            nc.sync.dma_start(out=x_g[g][:, :, :], in_=x_src[:, sl, :])
            nc.vector.tensor_mul(
                xsq_g[g][:, :, :], x_g[g][:, :, :], x_g[g][:, :, :]
            )
            nc.vector.tensor_reduce(
                out=s[:, sl],
                in_=xsq_g[g][:, :, :],
                axis=mybir.AxisListType.X,
                op=ALU.add,
            )
            # r = exp(-0.5*ln(s)) == s^-1/2 ; exp+ln share one ACT table set
            nc.scalar.activation(out=ls[:, sl], in_=s[:, sl], func=AF.Ln)
            nc.scalar.activation(
                out=r[:, sl], in_=ls[:, sl], func=AF.Exp, scale=-0.5
            )
            nc.vector.tensor_copy(r_dve[:, sl], r[:, sl])
            for c in range(8 * g, 8 * g + 8):
                nc.vector.tensor_scalar_mul(
                    out=xb[:, c, :],
                    in0=x_g[c // 8][:, c % 8, :],
                    scalar1=r_dve[:, c : c + 1],
                )
            # transpose completed pairs of groups (and the final odd group)
            if g % 2 == 1 or g == NGRP - 1:
                tg = g // 2
                pt = pts[tg]
                nchunks = 16 if g % 2 == 1 else 8
                for k in range(nchunks):
                    ch = 16 * tg + k
                    nc.tensor.transpose(
                        pt[:, 128 * k : 128 * (k + 1)], xb[:, ch, :], ident
                    )
                # copy on ACT: matmuls consuming xnT then wait on ACT only
                if tg == 0:
                    nc.scalar.copy(xnT0[:, :], pt[:, :])
                elif tg == 1:
                    nc.scalar.copy(xnT1[:, :], pt[:, :])
                else:
                    nc.scalar.copy(xnT2[:, :], pt[:, :])

    def xnT_col(j, width):
        """AP for xnT columns [j, j+width) -- must stay in one tile."""
        if j < 2048:
            assert j + width <= 2048
            return xnT0[:, j : j + width]
        if j < 4096:
            assert j + width <= 4096
            return xnT1[:, j - 2048 : j - 2048 + width]
        assert j + width <= 5120
        return xnT2[:, j - 4096 : j - 4096 + width]

    # ---- main loop: three ping-pong phases ----
    # A: [d0 d1] cols [0,2048);  B: [d2 d3] cols [2048,4096);  C: [d4]
    # cols [4096,5120).  Each phase double-buffers full-width PSUM tiles so
    # ACT streams gaplessly and PE always runs ahead.  All col-sums are
    # deferred to phase C (they read the retained e tiles), where 4 banks
    # are free.
    with tc.tile_pool(name="psAB", bufs=2, space="PSUM") as psAB:
        for ph, (base, ebuf) in enumerate([(0, eA), (2048, eB)]):
            for m in range(8):
                pm = psAB.tile([128, 2048], F32, tag="pm", name=f"pm{ph}_{m}")
                lhsT = xnT0[:, 128 * m : 128 * (m + 1)]
                for k in range(4):
                    nc.tensor.matmul(
                        pm[:, 512 * k : 512 * (k + 1)],
                        lhsT=lhsT,
                        rhs=xnT_col(base + 512 * k, 512),
                        start=True,
                        stop=True,
                    )
                nc.scalar.activation(
                    out=ebuf[:, m, :],
                    in_=pm[:, :],
                    func=AF.Exp,
                    scale=2.0,
                    accum_out=rsAB[:, 8 * ph + m : 8 * ph + m + 1],
                )
    with tc.tile_pool(name="psC", bufs=2, space="PSUM") as psC, \
            tc.tile_pool(name="psS", bufs=1, space="PSUM") as psS:
        # col-sum scratch: one [128,1] slot per (mtile, d, chunk); every
        # col-sum matmul is start&stop (PSUM allows only one PENDING
        # accumulation group per bank zero-region, so cross-instruction
        # accumulation is illegal)
        scr = psS.tile([128, 8, 32], F32, tag="scr")
        for m in range(8):
            pc = psC.tile([128, 1024], F32, tag="pc", name=f"pc_{m}")
            lhsT = xnT0[:, 128 * m : 128 * (m + 1)]
            for k in range(2):
                nc.tensor.matmul(
                    pc[:, 512 * k : 512 * (k + 1)],
                    lhsT=lhsT,
                    rhs=xnT_col(4096 + 512 * k, 512),
                    start=True,
                    stop=True,
                )
            nc.scalar.activation(
                out=e4a[:, m, :],
                in_=pc[:, :],
                func=AF.Exp,
                scale=2.0,
                accum_out=rsC[:, m : m + 1],
            )
            # d1/d2/d3 col sums for this m (eA/eB complete); d4 for m-1
            for d, esrc in ((0, eA[:, m, 1024:2048]), (1, eB[:, m, 0:1024]),
                            (2, eB[:, m, 1024:2048])):
                for ch in range(8):
                    nc.tensor.matmul(
                        scr[:, m, 8 * d + ch : 8 * d + ch + 1],
                        lhsT=esrc[:, 128 * ch : 128 * (ch + 1)],
                        rhs=onesb,
                        start=True,
                        stop=True,
                    )
            if m > 0:
                for ch in range(8):
                    nc.tensor.matmul(
                        scr[:, m - 1, 24 + ch : 24 + ch + 1],
                        lhsT=e4a[:, m - 1, 128 * ch : 128 * (ch + 1)],
                        rhs=halfb,
                        start=True,
                        stop=True,
                    )
        # d123 col-sum fold can start while PE finishes the last d4 sums
        nc.vector.tensor_reduce(
            out=csb[:, 0:24],
            in_=scr[:, :, 0:24].rearrange("p m c -> p c m"),
            axis=mybir.AxisListType.X,
            op=ALU.add,
        )
        for ch in range(8):
            nc.tensor.matmul(
                scr[:, 7, 24 + ch : 24 + ch + 1],
                lhsT=e4a[:, 7, 128 * ch : 128 * (ch + 1)],
                rhs=halfb,
                start=True,
                stop=True,
            )
        nc.vector.tensor_reduce(
            out=csb[:, 24:32],
            in_=scr[:, :, 24:32].rearrange("p m c -> p c m"),
            axis=mybir.AxisListType.X,
            op=ALU.add,
        )

    # rsA2[p,m] = (d0+d1) + (d2+d3) row sums
    nc.vector.tensor_reduce(
        out=rsA2,
        in_=rsAB.rearrange("p (k m) -> p m k", k=2),
        axis=mybir.AxisListType.X,
        op=ALU.add,
    )

    # ---- positive-pair term: sum over my rows of sim(i, i+N) ----
    # local pos column of local row i is always i + 4096 (rotation invariant)
    nc.vector.tensor_mul(pos_scr, xnT0[:, 0:RPC], xnT2[:, 0:RPC])
    nc.vector.tensor_reduce(
        out=possum, in_=pos_scr, axis=mybir.AxisListType.X, op=ALU.add
    )

    # ---- exchange: csb -> DRAM -> AllGather -> masked reduce ----
    nc.sync.dma_start(out=ag_in[:, :], in_=csb)
    nc.gpsimd.collective_compute(
        "AllGather",
        ALU.bypass,
        replica_groups=[list(range(NCORES))],
        ins=[ag_in[:, :].opt()],
        outs=[ag_out[:, :].opt()],
    )
    nc.sync.dma_start(
        out=gg, in_=ag_out[:, :].rearrange("(s p) c -> p s c", p=128)
    )
    # gm[p, s, d, ch] = gg * mask (mask broadcast along ch)
    nc.vector.tensor_mul(
        gm.rearrange("p (s d ch) -> p s d ch", s=8, d=4),
        gg.rearrange("p s (d ch) -> p s d ch", d=4),
        mask_t.rearrange("p (s d) -> p s d", s=8).broadcast_to([128, 8, 4, 8]),
    )
    # remote[p, m] = sum over (s,d) of gm  (s,d flatten to stride-8 x 32)
    nc.vector.tensor_reduce(
        out=remote,
        in_=gm.rearrange("p (sd ch) -> p ch sd", ch=8),
        axis=mybir.AxisListType.X,
        op=ALU.add,
    )

    # ---- finals ----
    # rt2 = rsA2 + 0.5*rsC  (d4 blocks are computed by both end cores)
    nc.vector.scalar_tensor_tensor(
        out=rt2, in0=rsC, scalar=0.5, in1=rsA2, op0=ALU.mult, op1=ALU.add
    )
    nc.vector.tensor_tensor(out=rt3, in0=rt2, in1=remote, op=ALU.add)
    # lg = ln(rowtotal - e^2), logsum = sum over the 8 Mtiles
    nc.scalar.activation(
        out=lg, in_=rt3, func=AF.Ln, bias=negE2[:, :], scale=1.0,
        accum_out=logsum,
    )
    # fin = logsum - 2 * possum
    nc.vector.scalar_tensor_tensor(
        out=fin,
        in0=possum,
        scalar=-2.0,
        in1=logsum,
        op0=ALU.mult,
        op1=ALU.add,
    )
    nc.scalar.copy(fin2, fin)  # ACT hop: final matmul waits on ACT only
    # partition reduce via ones-matmul
    with tc.tile_pool(name="psF", bufs=1, space="PSUM") as psF:
        pf = psF.tile([128, 512], F32, tag="pf")
        nc.tensor.matmul(
            pf[0:1, 0:1], lhsT=fin2, rhs=ones, start=True, stop=True
        )
        nc.vector.tensor_copy(fin_sb, pf[0:1, 0:1])
    # SWDGE for the tiny output write (direct-2D carries only one sync wait)
    nc.gpsimd.dma_start(out=out_ap, in_=fin_sb)
    if dbg_aps is not None:
        dbg_aps = dict(dbg_aps)
        stage = small.tile([128, 73], F32, tag="dbgstage")
        off = 0
        for t, w in [(rsA2, 8), (rsAB[:, 8:16], 8), (rsC, 8), (csb, 32),
                     (remote, 8), (rt3, 8), (possum, 1)]:
            nc.vector.tensor_copy(stage[:, off : off + w], t)
            off += w
        nc.gpsimd.dma_start(out=dbg_aps["stage"], in_=stage)


def _emit_colsums(nc, e123, e4, m, scr, onesb, halfb):
    """Column sums via lhsT=e-chunk, rhs=ones; one closed matmul per slot."""
    for d in range(3):          # d = 1,2,3 from e123
        for ch in range(8):
            nc.tensor.matmul(
                scr[:, m, 8 * d + ch : 8 * d + ch + 1],
                lhsT=e123[:, 1024 * d + 128 * ch : 1024 * d + 128 * (ch + 1)],
                rhs=onesb,
                start=True,
                stop=True,
            )
    for ch in range(8):         # d = 4 (half weight)
        nc.tensor.matmul(
            scr[:, m, 24 + ch : 24 + ch + 1],
            lhsT=e4[:, 128 * ch : 128 * (ch + 1)],
            rhs=halfb,
            start=True,
            stop=True,
        )


def _strip_self_waits(nc):
    """Drop engine-self semaphore waits from Matmult/Activation instructions.

    PE and ACT are strict in-order single queues whose semaphores increment
    at instruction completion in program order, so a wait on the engine's own
    semaphore is always transitively implied by queue order.  The Matmult
    instruction encoding only has room for ONE sync wait, so the extra
    self-wait breaks walrus codegen ("Too many sync wait commands").
    """
    eng_prefix = {
        mybir.EngineType.PE: "PE_",
        mybir.EngineType.Activation: "Activation_",
        mybir.EngineType.DVE: "DVE_",
    }
    for bb in nc.main_func.blocks:
        for ins in bb.instructions:
            si = ins.sync_info
            if si is None:
                continue
            if type(ins).__name__ == "InstDrain":
                # The tail drain's encoding carries only ONE sync wait.  The
                # output DMA's completion (DMASW, last in program order)
                # transitively implies the rest in the shipping build; debug
                # builds accept a rare stale 'out' in exchange for compiling.
                w = list(si.on_wait)
                sw = [x for x in w if (x.ant_name or "").startswith("DMASW")]
                if len(w) > 1 and sw:
                    si.on_wait = [sw[-1]]
                continue
            pfx = eng_prefix.get(getattr(ins, "engine", None))
            if pfx is None:
                continue
            w = list(si.on_wait)
            w2 = [x for x in w if not (x.ant_name or "").startswith(pfx)]
            if type(ins).__name__ == "InstMatmult":
                # Pool only produces the identity matrix here, and the DVE
                # probe-read of it precedes every DVE-produced matmul input,
                # so any Pool wait on a matmul is transitively covered by
                # its DVE wait.
                w2 = [x for x in w2 if not (x.ant_name or "").startswith("Pool_")]
            if len(w2) != len(w):
                si.on_wait = w2


def _build(strip: bool = True, debug_taps: bool = False):
    from contextlib import ExitStack

    nc = bass.Bass("TRN2", debug=False, num_devices=NCORES)
    x_in = nc.dram_tensor("x", [NLOAD, D], F32, kind="ExternalInput")
    mask_in = nc.dram_tensor("mask", [128, 32], F32, kind="ExternalInput")
    out = nc.dram_tensor("out", [1, 1], F32, kind="ExternalOutput")
    dbg_aps = None
    if debug_taps:
        dbg_aps = {
            "stage": nc.dram_tensor(
                "dbg", [128, 73], F32, kind="ExternalOutput"
            ).ap(),
            "e123": nc.dram_tensor(
                "dbg_e123", [128, 3072], BF16, kind="ExternalOutput"
            ).ap(),
            "e4": nc.dram_tensor(
                "dbg_e4", [128, 1024], BF16, kind="ExternalOutput"
            ).ap(),
        }
    with tile.TileContext(nc) as tc:
        with ExitStack() as ctx:
            _emit(tc, ctx, out.ap(), x_in.ap(), mask_in.ap(), dbg_aps)
    if strip:
        _strip_self_waits(nc)
    return nc


_NC_CACHE = None
_WARMED = False


def _get_nc():
    global _NC_CACHE
    if _NC_CACHE is None:
        _NC_CACHE = _build()
    return _NC_CACHE


def warmup(nc, in_maps):
    """First-ever execution races the NRT collective-comm init (the
    AllGather's data can land after its semaphore fires), so prime the
    comm path once; all warm executions are fully synchronized."""
    global _WARMED
    if not _WARMED:
        run_bass_kernel_spmd(nc, in_maps, core_ids=list(range(NCORES)))
        _WARMED = True


def make_in_maps(x: np.ndarray) -> list[dict]:
    """Per-core inputs: rolled+truncated x and the exchange-select mask."""
    in_maps = []
    for c in range(NCORES):
        xr = np.ascontiguousarray(
            np.roll(x, -RPC * c, axis=0)[:NLOAD], dtype=np.float32
        )
        mask = np.zeros((8, 4), dtype=np.float32)
        for d in range(1, 5):
            mask[(c - d) % NCORES, d - 1] = 1.0
        mask_t = np.ascontiguousarray(
            np.broadcast_to(mask.reshape(1, 32), (128, 32)), dtype=np.float32
        )
        in_maps.append({"x": xr, "mask": mask_t})
    return in_maps


def kernel(**inputs) -> np.ndarray:
    x = np.ascontiguousarray(
        np.asarray(inputs["projected_vectors"]), dtype=np.float32
    )
    assert x.shape == (N2, D)
    nc = _get_nc()
    in_maps = make_in_maps(x)
    warmup(nc, in_maps)
    res = run_bass_kernel_spmd(nc, in_maps, core_ids=list(range(NCORES)))
    total = np.float32(0.0)
    for rmap in res.results:
        total += np.float32(rmap["out"][0, 0])
    return np.asarray(total, dtype=np.float32)


if __name__ == "__main__":
    xt = np.random.randn(N2, D).astype(np.float32)
    print(kernel(projected_vectors=xt))
